# revision 1
# baseline (speedup 1.0000x reference)
"""Trainium2 Bass kernel for nn_DetectionCriterion (detection loss).

Data-parallel over the batch: 32 samples -> 4 per core x 8 cores.
Per sample (flat [128, 3200] tiles):
  E = exp(-cls*cm); mining = ln(1+E) = softplus(-cls*cm)
  cm' = (E >= e^0.03-1) * cm        (hard negative mining)
  balanced sampling: per-sign threshold t = 128th smallest noise among
  flagged entries, found exactly via per-partition max8 (the 128 global
  smallest spread ~Poisson(1)/partition; top-8 verified sufficient) then
  a 3-level 128-ary counting refinement over the 1024 candidates
  (bin width 2^-29; all edges exactly representable in f32).
  loss = sum(keep*mining) + 2*sum(kp*smooth_l1(reg-regmap)), with
  smooth_l1 masked via e=kp*d and decomposed as
  0.5e^2 - 0.5 relu(|e|-1)^2, summed by ACT accum_out.
Engine split: DVE elementwise/max8/counting, ACT exp/ln/|.|/relu/square+accum,
PE computes d=reg-rmap via +/-identity matmuls into PSUM and all
cross-partition reductions. Scalar partials summed across cores on host.
"""

import numpy as np
from contextlib import ExitStack

import concourse.bass as bass
import concourse.tile as tile
import concourse.mybir as mybir
from concourse.vector_clock import ScopedClock, VectorClock
from concourse.bass_utils import run_bass_kernel_spmd

FP = mybir.dt.float32
BF = mybir.dt.bfloat16
I32 = mybir.dt.int32
OP = mybir.AluOpType
AF = mybir.ActivationFunctionType

B, T, H, W = 32, 25, 128, 128
NCORES = 8
SPC = B // NCORES          # samples per core
P = 128
F = T * H * W // P         # 3200
FQ = F // 4                # 800 (reg quarter-chunk)
TAU_E = float(np.expm1(0.03))
W1 = 2.0 ** -15            # counting level widths (T0 = 2^-8, 128 edges/level)
W2 = 2.0 ** -22
W3 = 2.0 ** -29


def _flat128(ap):
    """[C,128,128] dram AP -> [128, C*128] partition-major contiguous."""
    return ap.rearrange("a h w -> (a h w)").rearrange("(p f) -> p f", p=P)


def _split_waits_in_bir(bir_json: bytes) -> bytes:
    """The walrus build here encodes at most ONE sem wait per instruction.
    Hoist excess waits onto injected same-engine Drain instructions placed
    immediately before the owning instruction."""
    import json as _json
    d = _json.loads(bir_json)
    ctr = 0
    for fn in d.get("functions", []):
        for blk in fn.get("blocks", []):
            new_insts = []
            for inst in blk.get("instructions", []):
                si = inst.get("sync_info")
                ow = si.get("on_wait") if si else None
                if ow and len(ow) > 1:
                    for w in ow[:-1]:
                        new_insts.append({
                            "engine": inst["engine"],
                            "ins": [],
                            "outs": [],
                            "name": f"I-wsplit{ctr}",
                            "opcode": "Drain",
                            "sync_info": {"on_update": [], "on_wait": [w]},
                        })
                        ctr += 1
                    si["on_wait"] = [ow[-1]]
                new_insts.append(inst)
            blk["instructions"] = new_insts
    return _json.dumps(d).encode()


_PATCHED = False


def _patch_compile_split_waits():
    global _PATCHED
    if _PATCHED:
        return
    _PATCHED = True
    import concourse.bass_utils as bu
    import concourse.bass2jax as b2j

    orig = bu.compile_bir_kernel

    def patched(bir_json, tmpdir, neff_name="file.neff"):
        return orig(_split_waits_in_bir(bir_json), tmpdir, neff_name=neff_name)

    bu.compile_bir_kernel = patched
    b2j.compile_bir_kernel = patched


class SplitDrainTileContext(tile.TileContext):
    """Tail drain split into single-wait drains (walrus here rejects several
    sem waits on one TPB_CTRL)."""

    def _drain_and_barrier(self, tick_clock, wait_clock):
        gc = tick_clock.global_clock
        ticks = list(gc)
        n = len(ticks)
        for i in range(n):
            if ticks[i] <= 0:
                continue
            vec = [0] * n
            vec[i] = ticks[i]
            d = self.nc.sync.drain()
            wait_clock.add_sem_waits(d.ins, ScopedClock({None: VectorClock(vec)}))
        self.nc.sync.drain()
        self.nc.all_engine_barrier()
        assert self.sems is not None
        popped = self.nc._tile_sem_poison_stack.pop()
        assert popped is self._sem_poison
        self.nc.clear_and_free_semaphores(list(self.sems.allocated().values()))
        self.nc.all_engine_barrier()


def build_program():
    nc = bass.Bass("TRN2", target_bir_lowering=False, debug=False)
    out_d = nc.dram_tensor("out", [1, 1], FP, kind="ExternalOutput")
    o_d = nc.dram_tensor("output", [SPC, 5 * T, H, W], FP, kind="ExternalInput")
    c_d = nc.dram_tensor("class_map", [SPC, T, H, W], I32, kind="ExternalInput")
    r_d = nc.dram_tensor("regression_map", [SPC, 4 * T, H, W], FP, kind="ExternalInput")
    n_d = nc.dram_tensor("noise", [SPC, T, H, W], FP, kind="ExternalInput")

    with SplitDrainTileContext(nc) as tc, ExitStack() as ctx:
        pio = ctx.enter_context(tc.tile_pool(name="pio", bufs=2))
        pmid = ctx.enter_context(tc.tile_pool(name="pmid", bufs=1))
        preg = ctx.enter_context(tc.tile_pool(name="preg", bufs=2))
        psm = ctx.enter_context(tc.tile_pool(name="psm", bufs=2))
        psel = ctx.enter_context(tc.tile_pool(name="psel", bufs=1))
        pacc = ctx.enter_context(tc.tile_pool(name="pacc", bufs=1))
        pconst = ctx.enter_context(tc.tile_pool(name="pconst", bufs=1))
        ppsV = ctx.enter_context(tc.tile_pool(name="ppsV", bufs=1, space="PSUM"))
        ppsD = ctx.enter_context(tc.tile_pool(name="ppsD", bufs=2, space="PSUM"))
        ppsB = ctx.enter_context(tc.tile_pool(name="ppsB", bufs=2, space="PSUM"))

        # ---- constants ----
        it32 = pconst.tile([P, 1], I32)
        nc.gpsimd.iota(it32[:], pattern=[[1, 1]], base=1, channel_multiplier=1)
        iota_f = pconst.tile([P, 1], FP)
        nc.vector.tensor_copy(iota_f[:], it32[:])
        ones_col = pconst.tile([P, 1], FP)
        nc.vector.memset(ones_col[:], 1.0)
        ones_row = pconst.tile([1, P], FP)
        nc.vector.memset(ones_row[:], 1.0)
        ident = pconst.tile([P, P], FP)
        nc.vector.memset(ident[:], 1.0)
        nc.gpsimd.affine_select(
            ident[:], ident[:], pattern=[[1, P]], compare_op=OP.is_equal,
            fill=0.0, base=0, channel_multiplier=-1,
        )
        nident = pconst.tile([P, P], FP)
        nc.vector.tensor_scalar_mul(nident[:], ident[:], -1.0)
        identb = pconst.tile([P, P], BF)
        nc.vector.tensor_copy(identb[:], ident[:])
        nidentb = pconst.tile([P, P], BF)
        nc.vector.tensor_copy(nidentb[:], nident[:])

        accC = pacc.tile([P, SPC], FP)    # + cls partials
        accE = pacc.tile([P, 16 * SPC], FP)   # + e^2 (or 2*huber) partials
        accR = pacc.tile([P, 16 * SPC], FP)   # - relu(|e|-1)^2 partials
        nc.vector.memset(accC[:], 0.0)
        nc.vector.memset(accE[:], 0.0)
        nc.vector.memset(accR[:], 0.0)

        def selection_negt(score):
            """score [128,F]: -noise for flagged else -1. Returns PSUM [128,1]
            holding -(t_up) broadcast, for keep = (score >= -t_up)."""
            m8 = psm.tile([P, 8], FP, tag="m8")
            nc.vector.max(m8[:], score[:])
            v8 = psm.tile([P, 8], FP, tag="v8")
            nc.vector.tensor_scalar_mul(v8[:], m8[:], -1.0)
            row = psel.tile([1, 1024], FP, tag="row")
            nc.sync.dma_start(row[:], v8[:])
            Vps = ppsV.tile([P, 1024], FP, tag="Vps")
            nc.tensor.matmul(Vps[:, 0:512], ones_row[:], row[:, 0:512],
                             start=True, stop=True)
            nc.tensor.matmul(Vps[:, 512:1024], ones_row[:], row[:, 512:1024],
                             start=True, stop=True)
            V = psel.tile([P, 1024], FP, tag="V")
            nc.scalar.copy(V[:], Vps[:])

            cjunk = psel.tile([P, 1024], FP, tag="cjunk")
            lo = None
            for lvl, wl in enumerate((W1, W2, W3)):
                edges = psm.tile([P, 1], FP, tag="edges")
                if lvl == 0:
                    nc.vector.tensor_scalar_mul(edges[:], iota_f[:], wl)
                else:
                    nc.vector.tensor_scalar(
                        edges[:], iota_f[:], wl, lob[:, 0:1], OP.mult, OP.add
                    )
                cnt = psm.tile([P, 1], FP, tag="cnt")
                nc.vector.tensor_scalar(
                    cjunk[:], V[:], edges[:, 0:1], None, OP.is_le, OP.add,
                    accum_out=cnt[:],
                )
                below = psm.tile([P, 1], FP, tag="below")
                nc.vector.tensor_scalar(below[:], cnt[:], 128.0, None, OP.is_lt)
                jst = ppsB.tile([1, 1], FP, tag="bc")
                nc.tensor.matmul(jst[:], below[:], ones_col[:], start=True, stop=True)
                lo_new = psm.tile([1, 1], FP, tag=f"lo{lvl}")
                if lvl == 0:
                    nc.vector.tensor_scalar(lo_new[:], jst[:], wl, None, OP.mult)
                else:
                    nc.vector.tensor_scalar(
                        lo_new[:], jst[:], wl, lo[:, 0:1], OP.mult, OP.add
                    )
                lo = lo_new
                if lvl < 2:
                    lob = ppsB.tile([P, 1], FP, tag="bc")
                    nc.tensor.matmul(lob[:], ones_row[:], lo[:], start=True, stop=True)
            # negt = -(lo3 + W3), broadcast via PE
            negt = psm.tile([1, 1], FP, tag="negt")
            nc.vector.tensor_scalar(negt[:], lo[:], W3, -1.0, OP.add, OP.mult)
            negtb = ppsB.tile([P, 1], FP, tag="bc")
            nc.tensor.matmul(negtb[:], ones_row[:], negt[:], start=True, stop=True)
            return negtb

        for s in range(SPC):
            # ---- load 25-channel tensors ----
            cls = pio.tile([P, F], FP, tag="cls")
            nc.sync.dma_start(cls[:], _flat128(o_d.ap()[s, 0:T]))
            cmi = pmid.tile([P, F], I32, tag="cmi")
            nc.sync.dma_start(cmi[:], _flat128(c_d.ap()[s]))
            noz = pio.tile([P, F], FP, tag="noz")
            nc.sync.dma_start(noz[:], _flat128(n_d.ap()[s]))

            # ---- mining ----
            negn = pmid.tile([P, F], FP, tag="negn")
            nc.scalar.mul(negn[:], noz[:], -1.0)
            cmf = pmid.tile([P, F], FP, tag="cmf")
            nc.scalar.activation(cmf[:], cmi[:], AF.Copy, bias=-1.0, scale=1.0)
            nc.vector.tensor_mul(cls[:], cls[:], cmf[:])           # x = cls*cm
            E = pmid.tile([P, F], FP, tag="E")
            nc.scalar.activation(E[:], cls[:], AF.Exp, scale=-1.0)  # e^-x
            mining = cls
            nc.scalar.activation(mining[:], E[:], AF.Ln, bias=ones_col[:])
            nc.vector.scalar_tensor_tensor(
                cmf[:], E[:], TAU_E, cmf[:], OP.is_ge, OP.mult      # cm' in-place
            )
            cmp_ = cmf

            # ---- selection (pos, neg) ----
            keeps = {}
            for sign, sval, seng in (("p", 1.0, nc.vector), ("n", -1.0, nc.vector)):
                bm = pmid.tile([P, F], FP, tag="bm")
                nc.vector.tensor_scalar(bm[:], cmp_[:], sval, 1.0, OP.is_equal, OP.subtract)
                score = pmid.tile([P, F], FP, tag=f"sc{sign}")
                seng.tensor_tensor(score[:], negn[:], bm[:], OP.min)
                keeps[sign] = (score, selection_negt(score))

            # kp needed standalone (reg mask); ktot = kp + kn fused
            score_p, negt_p = keeps["p"]
            score_n, negt_n = keeps["n"]
            kp = score_p
            nc.vector.tensor_scalar(kp[:], score_p[:], negt_p[:, 0:1], None, OP.is_ge)
            ktot = score_n
            nc.vector.scalar_tensor_tensor(
                ktot[:], score_n[:], negt_n[:, 0:1], kp[:], OP.is_ge, OP.add
            )
            # ---- cls loss partial: sum(ktot * mining) ----
            nc.vector.scalar_tensor_tensor(
                bm[:], ktot[:], 1.0, mining[:], OP.mult, OP.mult,
                accum_out=accC[:, s : s + 1],
            )

            # ---- reg loss: 4 blocks x 4 quarter-chunks ----
            # variant A (ACT huber): Sum e^2 - relu(|e|-1)^2 via ACT squares
            # variant B (Pool huber): Sum 2*huber = Sum t*(2e-t), t=clamp(e,+-1)
            for j in range(4):
                regb = preg.tile([P, F], BF, tag="regb")
                nc.gpsimd.dma_start(regb[:], _flat128(o_d.ap()[s, T + T * j : 2 * T + T * j]))
                rmapb = preg.tile([P, F], BF, tag="rmapb")
                nc.gpsimd.dma_start(rmapb[:], _flat128(r_d.ap()[s, T * j : T * (j + 1)]))
                for h in range(4):
                    ch = j * 4 + h
                    var_b = (ch % 4) == 3   # quarter of chunks on DVE-huber path
                    dps = ppsD.tile([P, 1024], FP, tag="dps")
                    for c0, c1 in ((0, 512), (512, FQ)):
                        nc.tensor.matmul(dps[:, c0:c1], identb[:],
                                         regb[:, h * FQ + c0 : h * FQ + c1],
                                         start=True, stop=False)
                        nc.tensor.matmul(dps[:, c0:c1], nidentb[:],
                                         rmapb[:, h * FQ + c0 : h * FQ + c1],
                                         start=False, stop=True)
                    e = preg.tile([P, FQ], FP, tag="e")
                    nc.vector.tensor_tensor(
                        e[:], dps[:, 0:FQ], kp[:, h * FQ : (h + 1) * FQ], OP.mult
                    )
                    if var_b:
                        t = preg.tile([P, FQ], FP, tag="t")
                        nc.vector.tensor_scalar(t[:], e[:], 1.0, -1.0, OP.min, OP.max)
                        u = preg.tile([P, FQ], FP, tag="u")
                        nc.vector.scalar_tensor_tensor(
                            u[:], e[:], 2.0, t[:], OP.mult, OP.subtract
                        )
                        nc.vector.scalar_tensor_tensor(
                            e[:], u[:], 1.0, t[:], OP.mult, OP.mult,
                            accum_out=accE[:, 16 * s + ch : 16 * s + ch + 1],
                        )
                    else:
                        a = preg.tile([P, FQ], FP, tag="a")
                        nc.scalar.activation(a[:], e[:], AF.Abs)
                        nc.scalar.activation(
                            e[:], e[:], AF.Square,
                            accum_out=accE[:, 16 * s + ch : 16 * s + ch + 1],
                        )
                        nc.vector.tensor_scalar(a[:], a[:], -1.0, 0.0, OP.add, OP.max)
                        nc.scalar.activation(
                            a[:], a[:], AF.Square,
                            accum_out=accR[:, 16 * s + ch : 16 * s + ch + 1],
                        )

        # ---- final: total = sum(accC) + sum(accE) - sum(accR) via PE ----
        sC = pacc.tile([P, 1], FP)
        nc.vector.tensor_reduce(sC[:], accC[:], axis=mybir.AxisListType.X, op=OP.add)
        sE = pacc.tile([P, 1], FP)
        nc.vector.tensor_reduce(sE[:], accE[:], axis=mybir.AxisListType.X, op=OP.add)
        sR = pacc.tile([P, 1], FP)
        nc.vector.tensor_reduce(sR[:], accR[:], axis=mybir.AxisListType.X, op=OP.add)
        nc.vector.tensor_add(sC[:], sC[:], sE[:])
        nc.vector.tensor_sub(sC[:], sC[:], sR[:])
        tot = ppsB.tile([1, 1], FP, tag="bc")
        nc.tensor.matmul(tot[:], sC[:], ones_col[:], start=True, stop=True)
        res = pacc.tile([1, 1], FP)
        nc.scalar.copy(res[:], tot[:])
        nc.sync.dma_start(out_d.ap(), res[:])

    return nc


def make_in_maps(output, class_map, regression_map, noise):
    in_maps = []
    for c in range(NCORES):
        sl = slice(c * SPC, (c + 1) * SPC)
        in_maps.append({
            "output": np.ascontiguousarray(output[sl]),
            "class_map": np.ascontiguousarray(class_map[sl]),
            "regression_map": np.ascontiguousarray(regression_map[sl]),
            "noise": np.ascontiguousarray(noise[sl]),
        })
    return in_maps


def kernel(output, class_map, regression_map, noise):
    _patch_compile_split_waits()
    nc = build_program()
    in_maps = make_in_maps(output, class_map, regression_map, noise)
    r = run_bass_kernel_spmd(nc, in_maps, list(range(NCORES)))
    total = np.float32(0.0)
    for c in range(NCORES):
        total = np.float32(total + r.results[c]["out"][0, 0])
    return np.float32(total)

